# revision 19
# baseline (speedup 1.0000x reference)
"""CoAttention kernel for 8 Trainium2 NeuronCores.

Problem: S, D: [8, 2048, 1024] f32.
  G_b = D_b @ S_b^T                         [2048, 2048]
  co_D = D + rowsoftmax(G) @ S
  co_S = S + rowsoftmax(G^T) @ D
Data-parallel over batch: one batch per core, same NEFF on all 8 cores.

Per-core algorithm (all loops fully unrolled, Tile-scheduled):
  Phase A: load S; keep S^T (fp32r, stage-1 rhs) and S_nat (fp16,
           stage-2 rhs) in SBUF.
  Phase B (per 128-row l-block): load D block, PE-transpose to fp32r
           D^T tiles; 8x accumulate fp32r matmuls -> G block (fp32);
           row max; exp (ACT, accum_out gives rowsum);
           O_D = (E/rowsum) @ S via PE-transposed fp16 E tiles
           + D residual -> co_D. Also: store the raw G block to DRAM
           and fold it into a running per-lane column-max (GpSimd).
  CM: one GpSimd partition_all_reduce turns the per-lane max into the
      column-max matrix CM (broadcast across partitions).
  Phase C (per 512-col m-slice group): reload G slices from DRAM,
           E2 = exp(G - CM) (DVE sub + ACT exp) into an SBUF-resident
           [l, m] fp16 array — already the right lhsT layout for
           O_S = (E2/colsum) @ D, so no transposes at all; colsum via
           an extra N=1 ones-matmul per weight load; + S residual
           -> co_S.
fp32 logits throughout; fp16 only on post-softmax attention weights
and stage-2 operands. Softmax uses exact per-row / per-column maxes.
"""

import numpy as np

P = 128
T = 2048
DH = 1024
LT = T // P     # 16 token blocks per side
KD = DH // P    # 8 contraction blocks
NTILE = 512     # matmul moving free dim
NCH = T // NTILE

DEFAULTS = dict(
    e_dtype="fp16",
    stage_bufs=3,
    gsb_bufs=2,
    epool_bufs=2,
    etp_bufs=3,
    outp_bufs=2,
    gpsum_bufs=2,
    opsum_bufs=1,
    tpsum_bufs=2,
    cg_bufs=4,
    opsum_c_bufs=3,
)

_CACHE = {}


def _build_nc(**overrides):
    import concourse.bass as bass
    import concourse.bass_isa as bass_isa
    import concourse.mybir as mybir
    import concourse.tile as tile
    from concourse import bacc
    from concourse.masks import make_identity

    p = dict(DEFAULTS)
    p.update(overrides)

    dt = mybir.dt
    f32, f32r = dt.float32, dt.float32r
    e16 = dt.float16 if p["e_dtype"] == "fp16" else dt.bfloat16
    AX = mybir.AxisListType.X
    EXP = mybir.ActivationFunctionType.Exp
    COPY = mybir.ActivationFunctionType.Copy
    MAX = mybir.AluOpType.max
    SUB = mybir.AluOpType.subtract

    nc = bacc.Bacc("TRN2", target_bir_lowering=False, debug=False)

    S_ap = nc.dram_tensor("S", [T, DH], f32, kind="ExternalInput").ap()
    D_ap = nc.dram_tensor("D", [T, DH], f32, kind="ExternalInput").ap()
    coD_ap = nc.dram_tensor("co_D", [T, DH], f32, kind="ExternalOutput").ap()
    coS_ap = nc.dram_tensor("co_S", [T, DH], f32, kind="ExternalOutput").ap()

    with tile.TileContext(nc) as tc:
        with (
            tc.tile_pool(name="consts", bufs=1) as consts,
            tc.tile_pool(name="keep", bufs=1) as keep,
            tc.tile_pool(name="dram", bufs=1, space="DRAM") as dram,
            tc.tile_pool(name="stage", bufs=p["stage_bufs"]) as stage,
            tc.tile_pool(name="outp", bufs=p["outp_bufs"]) as outp,
            tc.tile_pool(name="small", bufs=4) as small,
        ):
            ident_f32 = consts.tile([P, P], f32)
            make_identity(nc, ident_f32[:])
            ident_e16 = consts.tile([P, P], e16)
            make_identity(nc, ident_e16[:])
            ones_e16 = consts.tile([P, 1], e16)
            nc.vector.memset(ones_e16[:], 1.0)

            # persistent across B and C
            D_nat = keep.tile([P, LT, DH], e16)   # [l%128, (lblk, dcol)]
            Macc = keep.tile([P, T], f32)         # per-lane col-max partials
            CM = keep.tile([P, T], f32)           # broadcast column max
            nc.vector.memset(Macc[:], -3.0e38)

            Gdram = dram.tile([T, T], f32)        # raw G, [l, m]

            def emit_out(ps, rscale, resid, out_ap):
                o = outp.tile([P, DH], f32, tag="o", name="o")
                nc.scalar.activation(o[:], ps[:], COPY, scale=rscale[:])
                nc.gpsimd.tensor_add(o[:], o[:], resid[:])
                nc.gpsimd.dma_start(out_ap, o[:])

            # ================= Phase A + B scope =================
            bigB_ctx = tc.tile_pool(name="bigB", bufs=1)
            bigB = bigB_ctx.__enter__()
            S_T = bigB.tile([P, KD, T], f32r)     # [d%128, (dblk, m)]
            S_nat = bigB.tile([P, LT, DH], e16)   # [m%128, (mblk, dcol)]
            dtp_ctx = tc.tile_pool(name="dtp", bufs=2)
            dtp = dtp_ctx.__enter__()
            gsb_ctx = tc.tile_pool(name="gsb", bufs=p["gsb_bufs"])
            gsb = gsb_ctx.__enter__()
            epool_ctx = tc.tile_pool(name="epool", bufs=p["epool_bufs"])
            epool = epool_ctx.__enter__()
            etp_ctx = tc.tile_pool(name="etp", bufs=p["etp_bufs"])
            etp = etp_ctx.__enter__()
            tpsum_ctx = tc.tile_pool(name="tpsum", bufs=p["tpsum_bufs"], space="PSUM")
            tpsum = tpsum_ctx.__enter__()
            gpsum_ctx = tc.tile_pool(name="gpsum", bufs=p["gpsum_bufs"], space="PSUM")
            gpsum = gpsum_ctx.__enter__()
            opsum_ctx = tc.tile_pool(name="opsum", bufs=p["opsum_bufs"], space="PSUM")
            opsum = opsum_ctx.__enter__()

            # ---- Phase A: S loads -> S^T fp32r + S_nat ----
            st_tiles = {}
            for i in range(2):
                st_tiles[i] = stage.tile([P, DH], f32, tag="ld", name="st")
                nc.sync.dma_start(st_tiles[i][:], S_ap[i * P:(i + 1) * P, :])
            for i in range(LT):
                if i + 2 < LT:
                    st_tiles[i + 2] = stage.tile([P, DH], f32, tag="ld", name="st")
                    nc.sync.dma_start(
                        st_tiles[i + 2][:], S_ap[(i + 2) * P:(i + 3) * P, :]
                    )
                st = st_tiles.pop(i)
                nc.gpsimd.tensor_copy(S_nat[:, i, :], st[:])
                for g in range(2):
                    pt = tpsum.tile([P, 4, P], f32, tag="tp")
                    for k4 in range(4):
                        k = g * 4 + k4
                        nc.tensor.transpose(
                            pt[:, k4, :], st[:, k * P:(k + 1) * P], ident_f32[:]
                        )
                    nc.vector.tensor_copy(
                        S_T[:, g * 4:(g + 1) * 4, i * P:(i + 1) * P], pt[:]
                    )

            # ---- Phase B ----
            std_tiles = {}
            for i in range(2):
                std_tiles[i] = stage.tile([P, DH], f32, tag="ld", name="std")
                nc.sync.dma_start(std_tiles[i][:], D_ap[i * P:(i + 1) * P, :])
            for i in range(LT):
                if i + 2 < LT:
                    std_tiles[i + 2] = stage.tile([P, DH], f32, tag="ld", name="std")
                    nc.sync.dma_start(
                        std_tiles[i + 2][:], D_ap[(i + 2) * P:(i + 3) * P, :]
                    )
                std = std_tiles.pop(i)
                nc.gpsimd.tensor_copy(D_nat[:, i, :], std[:])
                dt_i = dtp.tile([P, KD, P], f32r, name="dt_i")
                for g in range(2):
                    pt = tpsum.tile([P, 4, P], f32, tag="tp")
                    for k4 in range(4):
                        k = g * 4 + k4
                        nc.tensor.transpose(
                            pt[:, k4, :], std[:, k * P:(k + 1) * P], ident_f32[:]
                        )
                    nc.vector.tensor_copy(dt_i[:, g * 4:(g + 1) * 4, :], pt[:])

                g_sb = gsb.tile([P, T], f32, name="g_sb")
                rmp = small.tile([P, NCH], f32, tag="rmp", name="rmp")
                for mc in range(NCH):
                    gp = gpsum.tile([P, NTILE], f32, tag="g", name="gp")
                    for k in range(KD):
                        nc.tensor.matmul(
                            gp[:],
                            dt_i[:, k, :],
                            S_T[:, k, mc * NTILE:(mc + 1) * NTILE],
                            start=(k == 0),
                            stop=(k == KD - 1),
                        )
                    nc.vector.tensor_copy(g_sb[:, mc * NTILE:(mc + 1) * NTILE], gp[:])
                    nc.vector.tensor_reduce(
                        rmp[:, mc:mc + 1],
                        g_sb[:, mc * NTILE:(mc + 1) * NTILE],
                        axis=AX, op=MAX,
                    )
                # raw G -> DRAM + running column-max partials
                nc.scalar.dma_start(Gdram[i * P:(i + 1) * P, :], g_sb[:])
                nc.vector.tensor_max(Macc[:], Macc[:], g_sb[:])

                # row softmax
                nr = small.tile([P, 1], f32, tag="nr", name="nr")
                nc.vector.reduce_max(nr[:], rmp[:], axis=AX, negate=True)
                e_i = epool.tile([P, T], e16, tag="e", name="e_i")
                rsp = small.tile([P, NCH], f32, tag="rsp", name="rsp")
                for mc in range(NCH):
                    sl = slice(mc * NTILE, (mc + 1) * NTILE)
                    nc.scalar.activation(
                        e_i[:, sl], g_sb[:, sl], EXP, bias=nr[:], scale=1.0,
                        accum_out=rsp[:, mc:mc + 1],
                    )
                rs = small.tile([P, 1], f32, tag="rs", name="rs")
                nc.vector.reduce_sum(rs[:], rsp[:], axis=AX)
                rrs = small.tile([P, 1], f32, tag="rrs", name="rrs")
                nc.vector.reciprocal(rrs[:], rs[:])

                # O_D = (E/rowsum) @ S
                od = opsum.tile([P, DH], f32, tag="o", name="od")
                for kg in range(4):
                    pte = tpsum.tile([P, 4, P], e16, tag="tpe")
                    for k4 in range(4):
                        kb = kg * 4 + k4
                        nc.tensor.transpose(
                            pte[:, k4, :], e_i[:, kb * P:(kb + 1) * P], ident_e16[:]
                        )
                    et = etp.tile([P, 4, P], e16, tag="et", name="et")
                    nc.vector.tensor_copy(et[:], pte[:])
                    for k4 in range(4):
                        kb = kg * 4 + k4
                        for n in range(DH // NTILE):
                            nc.tensor.matmul(
                                od[:, n * NTILE:(n + 1) * NTILE],
                                et[:, k4, :],
                                S_nat[:, kb, n * NTILE:(n + 1) * NTILE],
                                start=(kb == 0),
                                stop=(kb == LT - 1),
                            )
                emit_out(od, rrs, std, coD_ap[i * P:(i + 1) * P, :])

            opsum_ctx.__exit__(None, None, None)
            gpsum_ctx.__exit__(None, None, None)
            tpsum_ctx.__exit__(None, None, None)
            etp_ctx.__exit__(None, None, None)
            epool_ctx.__exit__(None, None, None)
            gsb_ctx.__exit__(None, None, None)
            dtp_ctx.__exit__(None, None, None)
            bigB_ctx.__exit__(None, None, None)

            # ================= Phase C =================
            nc.gpsimd.partition_all_reduce(
                CM[:], Macc[:], 128, bass_isa.ReduceOp.max
            )

            bigC_ctx = tc.tile_pool(name="bigC", bufs=1)
            bigC = bigC_ctx.__enter__()
            E2s = bigC.tile([P, LT, T], e16)      # [l%128, (lblk, m)]
            cg_ctx = tc.tile_pool(name="cgp", bufs=p["cg_bufs"])
            cgp = cg_ctx.__enter__()
            opc_ctx = tc.tile_pool(name="opsum_c", bufs=p["opsum_c_bufs"], space="PSUM")
            opsum_c = opc_ctx.__enter__()
            csp_ctx = tc.tile_pool(name="cspsum", bufs=2, space="PSUM")
            csp = csp_ctx.__enter__()

            sst_tiles = {}

            def load_sst(j):
                t_ = stage.tile([P, DH], f32, tag="ld", name="sst")
                nc.sync.dma_start(t_[:], S_ap[j * P:(j + 1) * P, :])
                sst_tiles[j] = t_

            load_sst(0)
            load_sst(1)

            for jg in range(NCH):
                # produce E2 slices [all lblk, this 512-wide m slice]
                for i in range(LT):
                    cg = cgp.tile([P, NTILE], f32, tag="cg", name="cg")
                    nc.sync.dma_start(
                        cg[:],
                        Gdram[i * P:(i + 1) * P, jg * NTILE:(jg + 1) * NTILE],
                    )
                    cs2 = cgp.tile([P, NTILE], f32, tag="cs2", name="cs2")
                    nc.vector.tensor_tensor(
                        out=cs2[:], in0=cg[:],
                        in1=CM[:, jg * NTILE:(jg + 1) * NTILE], op=SUB,
                    )
                    nc.scalar.activation(
                        E2s[:, i, jg * NTILE:(jg + 1) * NTILE], cs2[:],
                        EXP, bias=0.0, scale=1.0,
                    )
                # consume: 4 output m-blocks in this slice
                for j4 in range(4):
                    j = jg * 4 + j4
                    if j + 2 < T // P:
                        load_sst(j + 2)
                    osp = opsum_c.tile([P, DH], f32, tag="oc", name="osp")
                    cs_ps = csp.tile([P, 1], f32, tag="cs", name="cs_ps")
                    for i in range(LT):
                        lhsT = E2s[:, i, j * P:(j + 1) * P]
                        for n in range(DH // NTILE):
                            nc.tensor.matmul(
                                osp[:, n * NTILE:(n + 1) * NTILE],
                                lhsT,
                                D_nat[:, i, n * NTILE:(n + 1) * NTILE],
                                start=(i == 0),
                                stop=(i == LT - 1),
                            )
                        nc.tensor.matmul(
                            cs_ps[:], lhsT, ones_e16[:],
                            start=(i == 0), stop=(i == LT - 1),
                        )
                    rcs = small.tile([P, 1], f32, tag="rrs", name="rcs")
                    nc.vector.reciprocal(rcs[:], cs_ps[:])
                    emit_out(osp, rcs, sst_tiles.pop(j),
                             coS_ap[j * P:(j + 1) * P, :])

            csp_ctx.__exit__(None, None, None)
            opc_ctx.__exit__(None, None, None)
            cg_ctx.__exit__(None, None, None)
            bigC_ctx.__exit__(None, None, None)

    nc.compile()
    return nc


def _get_nc():
    if "nc" not in _CACHE:
        _CACHE["nc"] = _build_nc()
    return _CACHE["nc"]


def kernel(S, D):
    from concourse.bass_utils import run_bass_kernel_spmd

    S = np.ascontiguousarray(np.asarray(S, dtype=np.float32))
    D = np.ascontiguousarray(np.asarray(D, dtype=np.float32))
    B = S.shape[0]
    assert S.shape == (B, T, DH) and D.shape == (B, T, DH) and B == 8

    nc = _get_nc()
    in_maps = [{"S": S[b], "D": D[b]} for b in range(B)]
    res = run_bass_kernel_spmd(nc, in_maps, core_ids=list(range(B)))
    co_D = np.stack([res.results[b]["co_D"] for b in range(B)])
    co_S = np.stack([res.results[b]["co_S"] for b in range(B)])
    return (co_D, co_S)


# revision 23
# speedup vs baseline: 3.9878x; 3.9878x over previous
"""CoAttention kernel for 8 Trainium2 NeuronCores.

Problem: S, D: [8, 2048, 1024] f32.
  G_b = D_b @ S_b^T                         [2048, 2048]
  co_D = D + rowsoftmax(G) @ S
  co_S = S + rowsoftmax(G^T) @ D
Data-parallel over batch: one batch per core, same NEFF on all 8 cores.

Per-core algorithm (all loops fully unrolled, Tile-scheduled):
  Phase A: load S; keep S^T (fp32r, for stage-1 rhs) and S_nat (16-bit,
           stage-2 rhs) in SBUF.
  Phase B (per 128-row l-block): load D block, PE-transpose to fp32r
           D^T tiles; 8x accumulate fp32r matmuls -> G block; row max;
           exp (ACT, accum_out gives rowsum); PE-transpose the fp32 G
           block out to a DRAM G^T buffer for phase C; then
           O_D = (E/rowsum) @ S via PE-transposed 16-bit E tiles,
           + D residual -> co_D.
  Phase C (per 128-row m-block of G^T): load G^T rows, col max, exp,
           O_S = (E2/colsum) @ D via PE-transposed 16-bit E2 tiles,
           + S residual -> co_S.
fp32 logits throughout; 16-bit only on post-softmax attention weights
and stage-2 operands. Softmax uses exact per-row / per-column maxes.
"""

import numpy as np

P = 128
T = 2048
DH = 1024
LT = T // P     # 16 token blocks per side
KD = DH // P    # 8 contraction blocks
NTILE = 512     # matmul moving free dim

# tuning knobs (defaults = shipping config)
DEFAULTS = dict(
    e_dtype="fp16",       # dtype of E tiles / S_nat / D_nat (stage-2 operands)
    chunked_redmax=True,  # rowmax per G chunk instead of one big reduce
    chunked_exp=True,     # exp per 512-col chunk instead of one big activation
    use_gpsimd=True,      # casts + residual adds on GpSimd
    g_copy_eng="dve",     # engine for G psum->sbuf copies
    dma_transpose_e=False, # E-tile transposes on DMA xbar instead of PE+copy
    stage_bufs=3,
    gsb_bufs=2,
    epool_bufs=2,
    etp_bufs=3,
    gtsb_bufs=2,
    outp_bufs=2,
    gpsum_bufs=2,
    opsum_bufs=1,
    tpsum_bufs=2,
)

_CACHE = {}


def _build_nc(**overrides):
    import concourse.bass as bass
    import concourse.mybir as mybir
    import concourse.tile as tile
    from concourse import bacc
    from concourse.masks import make_identity

    p = dict(DEFAULTS)
    p.update(overrides)

    dt = mybir.dt
    f32, f32r = dt.float32, dt.float32r
    e16 = dt.float16 if p["e_dtype"] == "fp16" else dt.bfloat16
    AX = mybir.AxisListType.X
    EXP = mybir.ActivationFunctionType.Exp
    COPY = mybir.ActivationFunctionType.Copy
    MAX = mybir.AluOpType.max

    nc = bacc.Bacc("TRN2", target_bir_lowering=False, debug=False)

    S_ap = nc.dram_tensor("S", [T, DH], f32, kind="ExternalInput").ap()
    D_ap = nc.dram_tensor("D", [T, DH], f32, kind="ExternalInput").ap()
    coD_ap = nc.dram_tensor("co_D", [T, DH], f32, kind="ExternalOutput").ap()
    coS_ap = nc.dram_tensor("co_S", [T, DH], f32, kind="ExternalOutput").ap()

    NCH = T // NTILE  # 4 chunks per token row

    with tile.TileContext(nc) as tc:
        with (
            tc.tile_pool(name="consts", bufs=1) as consts,
            tc.tile_pool(name="big", bufs=1) as big,
            tc.tile_pool(name="dram", bufs=1, space="DRAM") as dram,
            tc.tile_pool(name="stage", bufs=p["stage_bufs"]) as stage,
            tc.tile_pool(name="epool", bufs=p["epool_bufs"]) as epool,
            tc.tile_pool(name="etp", bufs=max(6, p["etp_bufs"])) as etp,
            tc.tile_pool(name="gtsb", bufs=p["gtsb_bufs"]) as gtsb,
            tc.tile_pool(name="outp", bufs=p["outp_bufs"]) as outp,
            tc.tile_pool(name="small", bufs=4) as small,
            tc.tile_pool(name="tpsum", bufs=p["tpsum_bufs"], space="PSUM") as tpsum,
        ):
            ident_f32 = consts.tile([P, P], f32)
            make_identity(nc, ident_f32[:])
            ident_e16 = consts.tile([P, P], e16)
            make_identity(nc, ident_e16[:])

            S_T = big.tile([P, KD, T], f32r)      # [d%128, (dblk, m)]
            S_nat = big.tile([P, LT, DH], e16)    # [m%128, (mblk, dcol)]
            D_nat = big.tile([P, LT, DH], e16)    # [l%128, (lblk, dcol)]
            GT = dram.tile([T, T], f32)           # G^T in DRAM, [m, l]

            def softmax_row(g_sb, rmax_parts):
                """-rowmax -> exp -> E (e16) + rowsum + recip.

                Returns (e, rrs, ets): ets is the list of 4 transposed
                E-tile groups when the DMA-xbar path is on (each produced
                by an ACT-issued transpose DMA right after its exp chunk),
                else None."""
                nr = small.tile([P, 1], f32, tag="nr")
                if rmax_parts is not None:
                    nc.vector.reduce_max(nr[:], rmax_parts[:], axis=AX, negate=True)
                else:
                    nc.vector.reduce_max(nr[:], g_sb[:], axis=AX, negate=True)
                e = epool.tile([P, T], e16, tag="e")
                rs = small.tile([P, 1], f32, tag="rs")
                ets = [] if p["dma_transpose_e"] else None
                if p["chunked_exp"]:
                    rsp = small.tile([P, NCH], f32, tag="rsp", name="rsp")
                    for mc in range(NCH):
                        sl = slice(mc * NTILE, (mc + 1) * NTILE)
                        nc.scalar.activation(
                            e[:, sl], g_sb[:, sl], EXP, bias=nr[:], scale=1.0,
                            accum_out=rsp[:, mc:mc + 1],
                        )
                        if ets is not None:
                            et = etp.tile([P, 4, P], e16, tag="et", name="et")
                            nc.scalar.dma_start_transpose(et[:], e[:, sl])
                            ets.append(et)
                    nc.vector.reduce_sum(rs[:], rsp[:], axis=AX)
                else:
                    nc.scalar.activation(
                        e[:], g_sb[:], EXP, bias=nr[:], scale=1.0, accum_out=rs[:]
                    )
                    if ets is not None:
                        for mc in range(NCH):
                            et = etp.tile([P, 4, P], e16, tag="et", name="et")
                            nc.scalar.dma_start_transpose(
                                et[:], e[:, mc * NTILE:(mc + 1) * NTILE]
                            )
                            ets.append(et)
                rrs = small.tile([P, 1], f32, tag="rrs")
                nc.vector.reciprocal(rrs[:], rs[:])
                return e, rrs, ets

            def stage2(e, rhs_big, ps_tag, ets=None, pool=None):
                """O += E^T-tiles @ rhs over 16 K blocks. Returns psum [P, DH]."""
                ps = (pool or opsum).tile([P, DH], f32, tag=ps_tag, name="ps")
                for kg in range(4):
                    if ets is not None:
                        et = ets[kg]
                    else:
                        pte = tpsum.tile([P, 4, P], e16, tag="tpe")
                        for k4 in range(4):
                            kb = kg * 4 + k4
                            nc.tensor.transpose(
                                pte[:, k4, :], e[:, kb * P:(kb + 1) * P], ident_e16[:]
                            )
                        et = etp.tile([P, 4, P], e16, tag="et")
                        nc.vector.tensor_copy(et[:], pte[:])
                    for k4 in range(4):
                        kb = kg * 4 + k4
                        for n in range(DH // NTILE):
                            nc.tensor.matmul(
                                ps[:, n * NTILE:(n + 1) * NTILE],
                                et[:, k4, :],
                                rhs_big[:, kb, n * NTILE:(n + 1) * NTILE],
                                start=(kb == 0),
                                stop=(kb == LT - 1),
                            )
                return ps

            def emit_out(ps, rscale, resid, out_ap):
                o = outp.tile([P, DH], f32, tag="o")
                nc.scalar.activation(o[:], ps[:], COPY, scale=rscale[:])
                adder = nc.gpsimd if p["use_gpsimd"] else nc.vector
                adder.tensor_add(o[:], o[:], resid[:])
                dma_eng = nc.gpsimd if p["use_gpsimd"] else nc.sync
                dma_eng.dma_start(out_ap, o[:])

            # ---- Phase A: S loads, S^T fp32r + S_nat ----
            gpsum_ctx = tc.tile_pool(name="gpsum", bufs=p["gpsum_bufs"], space="PSUM")
            gpsum = gpsum_ctx.__enter__()
            opsum_ctx = tc.tile_pool(name="opsum", bufs=p["opsum_bufs"], space="PSUM")
            opsum = opsum_ctx.__enter__()
            ab_ctx = tc.tile_pool(name="dtp", bufs=2)
            dtp = ab_ctx.__enter__()
            gsb_ctx = tc.tile_pool(name="gsb", bufs=p["gsb_bufs"])
            gsb = gsb_ctx.__enter__()
            gtsb_ctx = tc.tile_pool(name="gtsb", bufs=p["gtsb_bufs"])
            gtsb = gtsb_ctx.__enter__()
            st_tiles = {}
            for i in range(2):
                st_tiles[i] = stage.tile([P, DH], f32, tag="ld", name="st")
                nc.sync.dma_start(st_tiles[i][:], S_ap[i * P:(i + 1) * P, :])
            for i in range(LT):
                if i + 2 < LT:
                    st_tiles[i + 2] = stage.tile([P, DH], f32, tag="ld", name="st")
                    nc.sync.dma_start(
                        st_tiles[i + 2][:], S_ap[(i + 2) * P:(i + 3) * P, :]
                    )
                st = st_tiles.pop(i)
                caster = nc.gpsimd if p["use_gpsimd"] else nc.vector
                caster.tensor_copy(S_nat[:, i, :], st[:])
                for g in range(2):
                    pt = tpsum.tile([P, 4, P], f32, tag="tp")
                    for k4 in range(4):
                        k = g * 4 + k4
                        nc.tensor.transpose(
                            pt[:, k4, :], st[:, k * P:(k + 1) * P], ident_f32[:]
                        )
                    nc.vector.tensor_copy(
                        S_T[:, g * 4:(g + 1) * 4, i * P:(i + 1) * P], pt[:]
                    )

            # ---- Phase B: G blocks, exp, G^T export, O_D ----
            std_tiles = {}
            for i in range(2):
                std_tiles[i] = stage.tile([P, DH], f32, tag="ld", name="std")
                nc.sync.dma_start(std_tiles[i][:], D_ap[i * P:(i + 1) * P, :])
            for i in range(LT):
                if i + 2 < LT:
                    std_tiles[i + 2] = stage.tile([P, DH], f32, tag="ld", name="std")
                    nc.sync.dma_start(
                        std_tiles[i + 2][:], D_ap[(i + 2) * P:(i + 3) * P, :]
                    )
                std = std_tiles.pop(i)
                caster = nc.gpsimd if p["use_gpsimd"] else nc.vector
                caster.tensor_copy(D_nat[:, i, :], std[:])
                dt_i = dtp.tile([P, KD, P], f32r)
                for g in range(2):
                    pt = tpsum.tile([P, 4, P], f32, tag="tp")
                    for k4 in range(4):
                        k = g * 4 + k4
                        nc.tensor.transpose(
                            pt[:, k4, :], std[:, k * P:(k + 1) * P], ident_f32[:]
                        )
                    nc.vector.tensor_copy(dt_i[:, g * 4:(g + 1) * 4, :], pt[:])

                g_sb = gsb.tile([P, T], f32)
                if p["chunked_redmax"]:
                    rmp = small.tile([P, NCH], f32, tag="rmp", name="rmp")
                else:
                    rmp = None
                for mc in range(NCH):
                    gp = gpsum.tile([P, NTILE], f32, tag="g")
                    for k in range(KD):
                        nc.tensor.matmul(
                            gp[:],
                            dt_i[:, k, :],
                            S_T[:, k, mc * NTILE:(mc + 1) * NTILE],
                            start=(k == 0),
                            stop=(k == KD - 1),
                        )
                    if p["g_copy_eng"] == "dve":
                        nc.vector.tensor_copy(g_sb[:, mc * NTILE:(mc + 1) * NTILE], gp[:])
                    else:
                        nc.scalar.copy(g_sb[:, mc * NTILE:(mc + 1) * NTILE], gp[:])
                    if rmp is not None:
                        nc.vector.tensor_reduce(
                            rmp[:, mc:mc + 1],
                            g_sb[:, mc * NTILE:(mc + 1) * NTILE],
                            axis=AX, op=MAX,
                        )
                e_i, rrs, ets = softmax_row(g_sb, rmp)

                # export G^T tiles to DRAM for phase C
                for g in range(4):
                    ptg = tpsum.tile([P, 4, P], f32, tag="tp")
                    for j4 in range(4):
                        j = g * 4 + j4
                        nc.tensor.transpose(
                            ptg[:, j4, :], g_sb[:, j * P:(j + 1) * P], ident_f32[:]
                        )
                    gt_sb = gtsb.tile([P, 4, P], f32)
                    nc.scalar.copy(gt_sb[:], ptg[:])
                    nc.scalar.dma_start(
                        GT[g * 4 * P:(g + 1) * 4 * P, i * P:(i + 1) * P].rearrange(
                            "(a p) c -> p a c", p=P
                        ),
                        gt_sb[:],
                    )

                od = stage2(e_i, S_nat, "o", ets)
                emit_out(od, rrs, std, coD_ap[i * P:(i + 1) * P, :])

            gtsb_ctx.__exit__(None, None, None)
            gsb_ctx.__exit__(None, None, None)
            ab_ctx.__exit__(None, None, None)
            opsum_ctx.__exit__(None, None, None)
            gpsum_ctx.__exit__(None, None, None)
            opsum_c_ctx = tc.tile_pool(name="opsum_c", bufs=2, space="PSUM")
            opsum_c = opsum_c_ctx.__enter__()
            gtld_ctx = tc.tile_pool(name="gtld", bufs=3)
            gtld = gtld_ctx.__enter__()

            # ---- Phase C: G^T rows -> col softmax -> O_S ----
            def load_c(j):
                g = gtld.tile([P, T], f32, name="gst")
                for mc in range(NCH):
                    nc.sync.dma_start(
                        g[:, mc * NTILE:(mc + 1) * NTILE],
                        GT[j * P:(j + 1) * P, mc * NTILE:(mc + 1) * NTILE],
                    )
                s = stage.tile([P, DH], f32, tag="ld", name="sst")
                nc.sync.dma_start(s[:], S_ap[j * P:(j + 1) * P, :])
                return g, s

            c_tiles = {}
            for j in range(2):
                c_tiles[j] = load_c(j)
            for j in range(LT):
                if j + 2 < LT:
                    c_tiles[j + 2] = load_c(j + 2)
                gst, sst = c_tiles.pop(j)
                cmp_ = small.tile([P, NCH], f32, tag="rmp", name="cmp_")
                for mc in range(NCH):
                    nc.vector.tensor_reduce(
                        cmp_[:, mc:mc + 1],
                        gst[:, mc * NTILE:(mc + 1) * NTILE],
                        axis=AX, op=MAX,
                    )
                e2, rcs, ets2 = softmax_row(gst, cmp_)
                os_ = stage2(e2, D_nat, "oc", ets2, pool=opsum_c)
                emit_out(os_, rcs, sst, coS_ap[j * P:(j + 1) * P, :])
            gtld_ctx.__exit__(None, None, None)
            opsum_c_ctx.__exit__(None, None, None)

    nc.compile()
    return nc


def _get_nc():
    if "nc" not in _CACHE:
        _CACHE["nc"] = _build_nc()
    return _CACHE["nc"]


def kernel(S, D):
    from concourse.bass_utils import run_bass_kernel_spmd

    S = np.ascontiguousarray(np.asarray(S, dtype=np.float32))
    D = np.ascontiguousarray(np.asarray(D, dtype=np.float32))
    B = S.shape[0]
    assert S.shape == (B, T, DH) and D.shape == (B, T, DH) and B == 8

    nc = _get_nc()
    in_maps = [{"S": S[b], "D": D[b]} for b in range(B)]
    res = run_bass_kernel_spmd(nc, in_maps, core_ids=list(range(B)))
    co_D = np.stack([res.results[b]["co_D"] for b in range(B)])
    co_S = np.stack([res.results[b]["co_S"] for b in range(B)])
    return (co_D, co_S)
